# revision 28
# baseline (speedup 1.0000x reference)
"""Trainium2 Bass kernel for CausalModulatedAttention.

Full-input contract: kernel(**inputs) takes the unsharded numpy inputs and
returns the full (B, L, D) float32 output.

Sharding: core = 2*b + g (b = batch, g = head-group).  The two cores of a
batch split the 16 heads (8 each) but both cover all 512 rows, with TRUE
causal extents per 128-row chunk (jext = 128*(ic+1)) -- no wasted score
columns.  The pairwise causal-graph bias G (shared by all heads) is instead
row-sharded across the pair -- core g computes G rows {0,3} or {1,2} chunks
(widths 256/512, SPMD-uniform) -- and the 192KB bias tiles are exchanged
through a per-pair AllGather.  Each core produces a partial output (its
heads' half of the d-contraction in the final projection); the host adds
the two halves.

Per core:
  - k/q/v projections in transposed layouts straight from x^T (PE),
    head-group slices only
  - pairwise G: gelu(he[j,c]+hc[i,c]+b1[c]) as one ACT op per 4-row group
    (per-partition bias), reduced over c via per-t stationary matrices (PE)
  - scores = q.k^T (PE); bias+mask tile added into PSUM on DVE
  - softmax without max-subtraction; Exp emits row sums via accum_out
  - transpose+normalize fused: EnT = E_chunk^T @ diag(1/rowsum) on PE
  - attn @ v on PE (col-group packed head pairs); partial output proj (PE)
All matmul operands bf16, fp32 PSUM accumulation.
"""

import math

import numpy as np
import ml_dtypes

import concourse.bass as bass
import concourse.mybir as mybir
import concourse.tile as tile
from concourse import bacc
from concourse.bass_utils import run_bass_kernel_spmd

BF = mybir.dt.bfloat16
F32 = mybir.dt.float32
AF = mybir.ActivationFunctionType
ALU = mybir.AluOpType

B, L, D = 4, 512, 1024
H, HD, CD = 16, 64, 32
ALPHA = 0.3
N_CORES = 8
HPC = 8               # heads per core
DPC = HPC * HD        # 512 d-columns per core
GJX = [256, 512]      # pairwise G width for (lo, hi) owned row chunk
NEG = -1.0e30
GW = GJX[0] + GJX[1]  # 768: packed G width per core
# packed bf16 consts: w2t (4096) | mask (768) | ident (128)
CPK = 4096 + GW + 128


def _bf(a):
    return np.ascontiguousarray(a.astype(ml_dtypes.bfloat16))


def _f32(a):
    return np.ascontiguousarray(a.astype(np.float32))


def core_rows(g):
    """Global row ranges (lo, hi) whose G rows core-group g computes."""
    lo = range(g * 128, g * 128 + 128)
    hi = range(384 - g * 128, 384 - g * 128 + 128)
    return lo, hi


def build_program():
    nc = bacc.Bacc("TRN2", num_devices=N_CORES, target_bir_lowering=False,
                   debug=False)

    boot_d = nc.dram_tensor("boot", [128, 1280], BF, kind="ExternalInput")
    xta_d = nc.dram_tensor("xta", [128, 8 * L], BF, kind="ExternalInput")
    xtb_d = nc.dram_tensor("xtb", [128, 8 * 256], BF, kind="ExternalInput")
    cpk_d = nc.dram_tensor("cpk", [128, CPK], BF, kind="ExternalInput")
    wk_d = nc.dram_tensor("wka", [128, 8 * DPC], BF, kind="ExternalInput")
    wq_d = nc.dram_tensor("wqa", [128, 8 * DPC], BF, kind="ExternalInput")
    wv_d = nc.dram_tensor("wva", [128, 8 * DPC], BF, kind="ExternalInput")
    wo_d = nc.dram_tensor("woa", [128, 4 * D], BF, kind="ExternalInput")
    b1_d = nc.dram_tensor("b1x4", [128, 1], F32, kind="ExternalInput")
    b2_d = nc.dram_tensor("b2h", [128, 1], F32, kind="ExternalInput")
    out_d = nc.dram_tensor("out", [L, D], BF, kind="ExternalOutput")

    with tile.TileContext(nc) as tc:
        with (
            tc.tile_pool(name="consts", bufs=1) as consts,
            tc.tile_pool(name="work", bufs=4) as work,
            tc.tile_pool(name="entp", bufs=6) as entp,
            tc.tile_pool(name="dram", bufs=1, space="DRAM") as dpool,
            tc.tile_pool(name="ppbig", bufs=4, space="PSUM") as ppbig,
            tc.tile_pool(name="pptp", bufs=2, space="PSUM") as pptp,
            tc.tile_pool(name="ppot", bufs=2, space="PSUM") as ppot,
        ):
            def load(name, shape, dt, src):
                t = consts.tile(shape, dt, tag=name)
                nc.sync.dma_start(out=t[:], in_=src)
                return t

            boot = load("boot", [128, 1280], BF, boot_d[:, :])
            xtb = load("xtb", [128, 8 * 256], BF, xtb_d[:, :])
            xta = load("xta", [128, 8 * L], BF, xta_d[:, :])
            cpk = load("cpk", [128, CPK], BF, cpk_d[:, :])
            b1x4 = load("b1x4", [128, 1], F32, b1_d[:, :])
            b2h = load("b2h", [128, 1], F32, b2_d[:, :])
            wka = load("wka", [128, 8 * DPC], BF, wk_d[:, :])
            wqa = load("wqa", [128, 8 * DPC], BF, wq_d[:, :])
            wva = load("wva", [128, 8 * DPC], BF, wv_d[:, :])
            woa = load("woa", [128, 4 * D], BF, wo_d[:, :])

            xT = [xta[:, mc * L:(mc + 1) * L] for mc in range(8)]
            xTq = [xtb[:, mc * 256:(mc + 1) * 256] for mc in range(8)]
            wc1 = boot[:, 0:256]
            we1x4 = boot[:, 256:1280]
            w2t = cpk[:, 0:4096]
            maskc = cpk[:, 4096:4096 + GW]
            ident = cpk[:, 4096 + GW:4096 + GW + 128]
            wk = [wka[:, mc * DPC:(mc + 1) * DPC] for mc in range(8)]
            wq = [wqa[:, mc * DPC:(mc + 1) * DPC] for mc in range(8)]
            wv = [wva[:, mc * DPC:(mc + 1) * DPC] for mc in range(8)]
            wo = [woa[:, dc * D:(dc + 1) * D] for dc in range(4)]

            # warm up the CC channel so the real exchanges are fast
            di = dpool.tile([128, 16], BF, tag="di")
            do = dpool.tile([2, 128, 16], BF, tag="do")
            dsb = consts.tile([128, 16], BF, tag="dsb")
            nc.gpsimd.memset(dsb[:], 0.0)
            nc.gpsimd.dma_start(out=di[:], in_=dsb[:])
            nc.gpsimd.collective_compute(
                "AllGather", ALU.bypass,
                replica_groups=[[0, 1], [2, 3], [4, 5], [6, 7]],
                ins=[di[:, :].opt()], outs=[do[:, :, :].opt()])

            # ---------- hc / he (unblock the gelu chain) ----------
            # hc4[u*32+c, oc*32+t] = (x @ Wc1)[oc*128+4t+u, c] + b1[c]
            # built directly on PE with a column-strided moving operand
            ps = ppbig.tile([128, 64], F32, tag="ps")
            for u in range(4):
                for mc in range(8):
                    rsrc = xTq[mc].rearrange("p (a t f) -> p a t f", a=2, f=4)[:, :, :, u]
                    nc.tensor.matmul(ps[u * CD:(u + 1) * CD, :],
                                     wc1[:, mc * CD:(mc + 1) * CD], rsrc,
                                     start=(mc == 0), stop=(mc == 7),
                                     tile_position=(0, u * CD))
            hc4 = consts.tile([128, 64], F32, tag="hc4")
            nc.vector.tensor_scalar_add(hc4[:], ps[:], b1x4[:, 0:1])

            ps = ppbig.tile([128, L], F32, tag="ps")
            for mc in range(8):
                nc.tensor.matmul(ps[:], we1x4[:, mc * 128:(mc + 1) * 128], xT[mc],
                                 start=(mc == 0), stop=(mc == 7))
            he4 = consts.tile([128, L], BF, tag="he4")
            nc.scalar.copy(he4[:], ps[:])

            # ---------- pairwise causal-graph bias (owned rows) ----------
            gsend = consts.tile([128, GW], BF, tag="gsend")

            def pairwise(oc):           # oc: owned chunk 0 (lo) / 1 (hi)
                jx = GJX[oc]
                moff = 0 if oc == 0 else GJX[0]
                graw = ppbig.tile([128, 512], F32, tag="ps")
                # true causal widths per 4-row group (rounded up to cover
                # both row-groups' SPMD-shared shape; columns beyond fd land
                # under the -inf mask).  t==0 spans the full width so
                # start=True clears has_written everywhere.
                fds = [jx if t == 0 else min(jx, (jx - 128) + 4 * t + 4)
                       for t in range(32)]
                for grp in range(8):
                    ts_ = list(range(grp * 4, grp * 4 + 4))
                    offs = [sum(fds[t] for t in ts_ if t < tt) for tt in ts_]
                    offs = [sum(fds[t2] for t2 in ts_[:k]) for k in range(4)]
                    tot = sum(fds[t] for t in ts_)
                    t4 = work.tile([128, tot], BF, tag=f"t4{oc}")
                    for k, t in enumerate(ts_):
                        nc.vector.tensor_scalar_add(
                            t4[:, offs[k]:offs[k] + fds[t]], he4[:, :fds[t]],
                            hc4[:, oc * 32 + t: oc * 32 + t + 1])
                    ga = work.tile([128, tot], BF, tag=f"ga{oc}")
                    nc.scalar.activation(ga[:], t4[:], AF.Gelu)
                    for k, t in enumerate(ts_):
                        nc.tensor.matmul(graw[:, :fds[t]],
                                         w2t[:, t * 128:(t + 1) * 128],
                                         ga[:, offs[k]:offs[k] + fds[t]],
                                         start=(t == 0), stop=(t == 31))
                th = work.tile([128, jx], BF, tag=f"th{oc}")
                nc.scalar.activation(th[:], graw[:, :jx], AF.Tanh, scale=0.5,
                                     bias=b2h[:, 0:1])
                nc.vector.scalar_tensor_tensor(
                    gsend[:, moff:moff + jx], th[:], ALPHA / 2.0,
                    maskc[:, moff:moff + jx], op0=ALU.mult, op1=ALU.add)

            # two-phase exchange: hi chunks first so the wide attention
            # work unblocks while the lo gelus still run
            gmap = {}

            def exchange(oc):
                jx = GJX[oc]
                moff = 0 if oc == 0 else GJX[0]
                gin = dpool.tile([128, jx], BF, tag=f"gin{oc}")
                gout = dpool.tile([2, 128, jx], BF, tag=f"gout{oc}")
                nc.sync.dma_start(out=gin[:], in_=gsend[:, moff:moff + jx])
                nc.gpsimd.collective_compute(
                    "AllGather", ALU.bypass,
                    replica_groups=[[0, 1], [2, 3], [4, 5], [6, 7]],
                    ins=[gin[:, :].opt()], outs=[gout[:, :, :].opt()])
                ga_ = consts.tile([128, jx], BF, tag=f"gx{oc}0")
                gb_ = consts.tile([128, jx], BF, tag=f"gx{oc}1")
                nc.sync.dma_start(out=ga_[:], in_=gout[0, :, :])
                nc.sync.dma_start(out=gb_[:], in_=gout[1, :, :])
                # G rows {0,3} came from rank 0, {1,2} from rank 1
                if oc == 0:
                    gmap[0], gmap[1] = ga_, gb_
                else:
                    gmap[3], gmap[2] = ga_, gb_

            pairwise(1)
            exchange(1)
            pairwise(0)
            exchange(0)

            # ---------- projection emitters ----------
            kT, qT, v = [None] * 4, [None] * 4, [None] * 4

            def proj_kq(dc):
                ps = ppbig.tile([128, L], F32, tag="ps")
                for mc in range(8):
                    nc.tensor.matmul(ps[:], wk[mc][:, dc * 128:(dc + 1) * 128],
                                     xT[mc], start=(mc == 0), stop=(mc == 7))
                t = consts.tile([128, L], BF, tag=f"kT{dc}")
                nc.vector.tensor_copy(t[:], ps[:])
                kT[dc] = t
                ps = ppbig.tile([128, L], F32, tag="ps")
                for mc in range(8):
                    nc.tensor.matmul(ps[:], wq[mc][:, dc * 128:(dc + 1) * 128],
                                     xT[mc], start=(mc == 0), stop=(mc == 7))
                t = consts.tile([128, L], BF, tag=f"qT{dc}")
                nc.vector.tensor_copy(t[:], ps[:])
                qT[dc] = t

            def proj_v(jc):
                t = consts.tile([128, DPC], BF, tag=f"v{jc}")
                ps = ppbig.tile([128, DPC], F32, tag="ps")
                for mc in range(8):
                    nc.tensor.matmul(ps[:], xT[mc][:, jc * 128:(jc + 1) * 128],
                                     wv[mc], start=(mc == 0), stop=(mc == 7))
                nc.vector.tensor_copy(t[:], ps[:])
                v[jc] = t

            # ---------- attention ----------
            ot = [[None] * 4 for _ in range(4)]

            def attention(ic, hp):
                jx = 128 * (ic + 1)
                njc = ic + 1
                gt = gmap[ic]
                otp = ppot.tile([128, 128], F32, tag="otp")
                for sub in range(2):
                    h = 2 * hp + sub
                    po = 64 * sub
                    sc = ppbig.tile([128, 512], F32, tag="ps")
                    nc.tensor.matmul(
                        sc[:, :jx], qT[hp][po:po + 64, ic * 128:(ic + 1) * 128],
                        kT[hp][po:po + 64, :jx], start=True, stop=False,
                        tile_position=(po, 0))
                    nc.tensor.matmul(sc[:, :jx], ident, gt[:, :jx],
                                     start=False, stop=True)
                    e = work.tile([128, jx], BF, tag=f"e{ic}")
                    sums = work.tile([128, 1], F32, tag="sums")
                    nc.scalar.activation(e[:], sc[:, :jx], AF.Exp,
                                         accum_out=sums[:, 0:1])
                    inv = work.tile([128, 1], F32, tag="inv")
                    nc.vector.reciprocal(inv[:], sums[:])
                    dg = work.tile([128, 128], BF, tag="dg")
                    nc.vector.tensor_scalar_mul(dg[:], ident, inv[:, 0:1])
                    for jc in range(njc):
                        etp = pptp.tile([128, 128], F32, tag="etp")
                        nc.tensor.matmul(etp[:], e[:, jc * 128:(jc + 1) * 128],
                                         dg[:], start=True, stop=True)
                        ent = entp.tile([128, 128], BF, tag="ent")
                        if jc % 4 == 3:
                            nc.scalar.copy(ent[:], etp[:])
                        else:
                            nc.vector.tensor_copy(ent[:], etp[:])
                        nc.tensor.matmul(
                            otp[po:po + 64, :], v[jc][:, h * HD:(h + 1) * HD],
                            ent[:], start=(jc == 0), stop=(jc == njc - 1),
                            tile_position=(0, po))
                t = consts.tile([128, 128], BF, tag=f"ot{ic}_{hp}")
                nc.vector.tensor_copy(t[:], otp[:])
                ot[ic][hp] = t

            def out_proj(ic, nn):
                ps = ppbig.tile([128, 512], F32, tag="ps")
                for dc in range(4):
                    nc.tensor.matmul(ps[:], ot[ic][dc][:],
                                     wo[dc][:, nn * 512:(nn + 1) * 512],
                                     start=(dc == 0), stop=(dc == 3))
                osb = work.tile([128, 512], BF, tag="osb")
                nc.vector.tensor_copy(osb[:], ps[:])
                nc.sync.dma_start(
                    out=out_d[ic * 128:(ic + 1) * 128, nn * 512:(nn + 1) * 512],
                    in_=osb[:])

            # emission: projections, then attention largest-first with the
            # finished chunks' output projections interleaved as PE fillers
            proj_kq(0)
            proj_v(0)
            proj_v(1)
            proj_kq(1)
            proj_v(2)
            proj_v(3)
            attention(3, 0)
            proj_kq(2)
            attention(3, 1)
            proj_kq(3)
            attention(3, 2)
            attention(3, 3)
            for hp in range(4):
                attention(2, hp)
                if hp >= 2:
                    out_proj(3, hp - 2)
            for hp in range(4):
                attention(1, hp)
                if hp >= 2:
                    out_proj(2, hp - 2)
            for hp in range(4):
                attention(0, hp)
                if hp >= 2:
                    out_proj(1, hp - 2)
            out_proj(0, 0)
            out_proj(0, 1)

    nc.compile()
    return nc


def _host_inputs(x, Wq, Wk, Wv, Wo, Wc, We, W1c, W1e, b1, W2, b2):
    """Per-core input dicts (host-side shard/cast/pack)."""
    x = _f32(np.asarray(x))
    wq_s = _f32(np.asarray(Wq) / math.sqrt(HD))
    wk = _f32(np.asarray(Wk))
    wv = _f32(np.asarray(Wv))
    wo = _f32(np.asarray(Wo))
    wc1 = _f32(np.asarray(Wc) @ np.asarray(W1c))      # (D, CD)
    we1 = _f32(np.asarray(We) @ np.asarray(W1e))
    wc1r = wc1.reshape(8, 128, CD).transpose(1, 0, 2).reshape(128, 8 * CD)
    we1c = we1.reshape(8, 128, CD).transpose(1, 0, 2)          # (128, 8, CD)
    we1x4 = np.tile(we1c[:, :, None, :], (1, 1, 4, 1)).reshape(128, 8 * 128)
    b1x4 = _f32(np.tile(np.asarray(b1).reshape(1, CD), (4, 1)).reshape(128, 1))
    b2h = _f32(np.full((128, 1), 0.5 * float(np.asarray(b2).reshape(-1)[0])))
    w2 = _f32(np.asarray(W2))

    # w2t[p=u*32+c, t*128 + m] = W2[c] if m == 4t+u else 0
    w2t = np.zeros((32, 128, 128), np.float32)
    for t in range(32):
        for u in range(4):
            w2t[t, u * CD:(u + 1) * CD, 4 * t + u] = w2
    w2t = w2t.transpose(1, 0, 2).reshape(128, 32 * 128)

    identb = np.eye(128, dtype=np.float32)
    bootc = np.concatenate([wc1r, we1x4], axis=1)

    def hpack(w, cols):  # (1024, cols) -> (128, 8*cols) m-chunk-major
        return w.reshape(8, 128, cols).transpose(1, 0, 2).reshape(128, 8 * cols)

    in_maps = []
    for core in range(N_CORES):
        b, g = core // 2, core % 2
        lo, hi = core_rows(g)
        rows = np.concatenate([np.arange(lo.start, lo.stop),
                               np.arange(hi.start, hi.stop)])
        hd0 = g * DPC                                  # head-group d offset
        xTb = np.ascontiguousarray(x[b].T)             # (D, L)
        mask = np.zeros((128, GW), np.float32)
        moff = 0
        for oc, rng in enumerate((lo, hi)):
            jx = GJX[oc]
            jj = np.arange(jx)[None, :]
            rr = np.arange(rng.start, rng.stop)[:, None]
            mask[:, moff:moff + jx] = np.where(jj <= rr, 0.0, NEG)
            moff += jx
        xTb8 = hpack(xTb, L)
        xTq8 = hpack(np.ascontiguousarray(xTb[:, rows]), 256)
        in_maps.append({
            "boot": _bf(bootc),
            "xta": _bf(xTb8), "xtb": _bf(xTq8),
            "cpk": _bf(np.concatenate([w2t, mask, identb], axis=1)),
            "wka": _bf(hpack(wk[:, hd0:hd0 + DPC], DPC)),
            "wqa": _bf(hpack(wq_s[:, hd0:hd0 + DPC], DPC)),
            "wva": _bf(hpack(wv[:, hd0:hd0 + DPC], DPC)),
            "woa": _bf(np.ascontiguousarray(
                wo[hd0:hd0 + DPC].reshape(4, 128, D)
                .transpose(1, 0, 2).reshape(128, 4 * D))),
            "b1x4": b1x4, "b2h": b2h,
        })
    return in_maps


def run(inputs: dict, trace: bool = False):
    """Build, run on 8 cores, return (full_output, BassKernelResults)."""
    nc = build_program()
    in_maps = _host_inputs(**inputs)
    res = run_bass_kernel_spmd(nc, in_maps, core_ids=list(range(N_CORES)),
                               trace=trace)
    out = np.zeros((B, L, D), np.float32)
    for b in range(B):
        out[b] = (res.results[2 * b]["out"].astype(np.float32)
                  + res.results[2 * b + 1]["out"].astype(np.float32))
    return out, res


def kernel(**inputs) -> np.ndarray:
    out, _ = run(inputs, trace=False)
    return out
